# revision 2
# baseline (speedup 1.0000x reference)
"""Squeeze-and-Excitation attention module on 8 Trainium2 NeuronCores.

Reference computation (per image b):
    y[c]  = mean(x[b, c, :, :])                      # global average pool
    z     = relu(w1 @ y + b1)                        # FC 512 -> 32
    s     = sigmoid(w2 @ z + b2)                     # FC 32 -> 512
    out[b, c, :, :] = x[b, c, :, :] * s[c]

Sharding: data-parallel over batch. 32 images / 8 cores = 4 images per
core; the tiny FC weights are replicated.

The kernel is HBM-bandwidth bound (read x once, write out once), so the
bulk data moves as fp16: the host casts x to fp16 (and the output back
to f32). That halves HBM traffic vs f32 — 2 x 16.8 MB per core instead
of 2 x 33.5 MB — while the pooled sums, FC gate math, and scale factors
stay in f32 on-chip. Quantization error is ~5e-4 rms, well inside the
2e-2 gate.

Layouts (prepared host-side):
    x      [4, 128, 4, 4096]  per-core shard, fp16. Channel c = k*128+p
                               lives at [b, p, k, :]; spatial flattened.
    w1t    [128, 4, 32]  f32   w1t[p, k, r] = w1[r, 128k + p]
    b1     [32, 1]       f32
    w2t    [32, 4, 128]  f32   w2t[r, k, p] = w2[128k + p, r]
    b2c    [128, 4]      f32   b2c[p, k]   = b2[128k + p]

Per image one SBUF tile [128, 4, 4096] fp16 (1 MB/half-image DMA
granularity: loads land per half, pooling tracks halves). All four
images fit in SBUF simultaneously so loads never wait on slot reuse.
Loads ride the Sync HWDGE queue, stores the GpSimd SWDGE queue (a store
waiting on compute never head-of-line-blocks the next load). Pooling
runs on DVE; scale multiplies split ACT/DVE.
"""

import numpy as np

B = 32
C = 512
HW = 64 * 64
N_CORES = 8
B_LOC = B // N_CORES
KC = C // 128  # channel chunks of 128

_NC_CACHE = {}

# Set by test harness to capture a profile; harmless default for grading.
TRACE = False
LAST_RESULT = None


def _build_nc():
    from contextlib import ExitStack

    import concourse.tile as tile
    from concourse import bacc, mybir

    f32 = mybir.dt.float32
    f16 = mybir.dt.float16
    AF = mybir.ActivationFunctionType
    nc = bacc.Bacc("TRN2", target_bir_lowering=False, debug=False)

    x = nc.dram_tensor("x", [B_LOC, 128, KC, HW], f16, kind="ExternalInput")
    w1t = nc.dram_tensor("w1t", [128, KC, 32], f32, kind="ExternalInput")
    b1 = nc.dram_tensor("b1", [32, 1], f32, kind="ExternalInput")
    w2t = nc.dram_tensor("w2t", [32, KC, 128], f32, kind="ExternalInput")
    b2c = nc.dram_tensor("b2c", [128, KC], f32, kind="ExternalInput")
    out = nc.dram_tensor("out", [B_LOC, 128, KC, HW], f16, kind="ExternalOutput")

    with ExitStack() as ctx:
        tc = ctx.enter_context(tile.TileContext(nc))
        singles = ctx.enter_context(tc.tile_pool(name="singles", bufs=1))
        xpool = ctx.enter_context(tc.tile_pool(name="xpool", bufs=B_LOC))
        small = ctx.enter_context(tc.tile_pool(name="small", bufs=2))
        psum = ctx.enter_context(tc.tile_pool(name="psum", bufs=2, space="PSUM"))

        w1t_sb = singles.tile([128, KC, 32], f32)
        b1_sb = singles.tile([32, 1], f32)
        w2t_sb = singles.tile([32, KC, 128], f32)
        b2_sb = singles.tile([128, KC], f32)

        for b in range(B_LOC):
            # Whole image in one tile; loads land as two 2 MB halves so
            # pooling starts when the first half arrives.
            xt = xpool.tile([128, KC, HW], f16, tag="x")
            nc.sync.dma_start(out=xt[:, 0:2, :], in_=x[b, :, 0:2, :])
            nc.sync.dma_start(out=xt[:, 2:4, :], in_=x[b, :, 2:4, :])

            if b == 0:
                # Weight loads ride the otherwise-idle SWDGE queue so
                # they never delay image loads on the Sync ring.
                nc.gpsimd.dma_start(out=w1t_sb, in_=w1t[:])
                nc.gpsimd.dma_start(out=b1_sb, in_=b1[:])
                nc.gpsimd.dma_start(out=w2t_sb, in_=w2t[:])
                nc.gpsimd.dma_start(out=b2_sb, in_=b2c[:])

            # Global average pool (sums; the 1/HW fold happens in the
            # relu activation's scale).
            sums = small.tile([128, KC], f32, tag="sums")
            nc.vector.tensor_reduce(
                out=sums[:, 0:2],
                in_=xt[:, 0:2, :],
                axis=mybir.AxisListType.X,
                op=mybir.AluOpType.add,
            )
            nc.vector.tensor_reduce(
                out=sums[:, 2:4],
                in_=xt[:, 2:4, :],
                axis=mybir.AxisListType.X,
                op=mybir.AluOpType.add,
            )

            zp = psum.tile([32, 1], f32, tag="z")
            for k in range(KC):
                nc.tensor.matmul(
                    zp,
                    lhsT=w1t_sb[:, k, :],
                    rhs=sums[:, k : k + 1],
                    start=(k == 0),
                    stop=(k == KC - 1),
                )

            z = small.tile([32, 1], f32, tag="z_sb")
            nc.scalar.activation(z, zp, AF.Relu, bias=b1_sb, scale=1.0 / HW)

            sp = psum.tile([128, KC], f32, tag="s")
            for k in range(KC):
                nc.tensor.matmul(
                    sp[:, k : k + 1],
                    lhsT=w2t_sb[:, k, :],
                    rhs=z,
                    start=True,
                    stop=True,
                )
            s = small.tile([128, KC], f32, tag="s_sb")
            for k in range(KC):
                nc.scalar.activation(
                    s[:, k : k + 1], sp[:, k : k + 1], AF.Sigmoid,
                    bias=b2_sb[:, k : k + 1],
                )

            # Scale in place and store per half-image. ACT takes the
            # first half, DVE the second (DVE also pools, ACT also runs
            # the gate's activations — roughly balanced, both well under
            # the DMA budget).
            nc.scalar.mul(xt[:, 0, :], xt[:, 0, :], s[:, 0:1])
            nc.scalar.mul(xt[:, 1, :], xt[:, 1, :], s[:, 1:2])
            nc.gpsimd.dma_start(out=out[b, :, 0:2, :], in_=xt[:, 0:2, :])
            nc.vector.tensor_scalar_mul(xt[:, 2, :], xt[:, 2, :], s[:, 2:3])
            nc.vector.tensor_scalar_mul(xt[:, 3, :], xt[:, 3, :], s[:, 3:4])
            nc.gpsimd.dma_start(out=out[b, :, 2:4, :], in_=xt[:, 2:4, :])

    nc.compile()
    return nc


def _get_nc():
    if "nc" not in _NC_CACHE:
        _NC_CACHE["nc"] = _build_nc()
    return _NC_CACHE["nc"]


def kernel(x, w1, b1, w2, b2):
    global LAST_RESULT
    from concourse.bass_utils import run_bass_kernel_spmd

    # [B, C, 64, 64] f32 -> [B, 128, KC, HW] fp16 with c = k*128 + p.
    xf = np.ascontiguousarray(
        x.astype(np.float16).reshape(B, KC, 128, HW).transpose(0, 2, 1, 3)
    )
    w1t = np.ascontiguousarray(w1.reshape(32, KC, 128).transpose(2, 1, 0))
    b1c = np.ascontiguousarray(b1.reshape(32, 1))
    w2t = np.ascontiguousarray(w2.reshape(KC, 128, 32).transpose(2, 0, 1))
    b2c = np.ascontiguousarray(b2.reshape(KC, 128).T)

    in_maps = [
        {
            "x": np.ascontiguousarray(xf[i * B_LOC : (i + 1) * B_LOC]),
            "w1t": w1t,
            "b1": b1c,
            "w2t": w2t,
            "b2c": b2c,
        }
        for i in range(N_CORES)
    ]

    nc = _get_nc()
    res = run_bass_kernel_spmd(
        nc, in_maps, core_ids=list(range(N_CORES)), trace=TRACE
    )
    LAST_RESULT = res
    out = np.concatenate([r["out"] for r in res.results], axis=0)
    # [B, 128, KC, HW] -> [B, KC, 128, HW] == [B, C, 64, 64]
    return (
        out.transpose(0, 2, 1, 3).reshape(B, C, 64, 64).astype(np.float32)
    )


# revision 15
# speedup vs baseline: 1.3052x; 1.3052x over previous
"""Squeeze-and-Excitation attention module on 8 Trainium2 NeuronCores.

Reference computation (per image b):
    y[c]  = mean(x[b, c, :, :])                      # global average pool
    z     = relu(w1 @ y + b1)                        # FC 512 -> 32
    s     = sigmoid(w2 @ z + b2)                     # FC 32 -> 512
    out[b, c, :, :] = x[b, c, :, :] * s[c]

Sharding: data-parallel over batch. 32 images / 8 cores = 4 images per
core; the tiny FC weights are replicated.

The kernel is HBM-bandwidth bound (read x once, write out once), so the
bulk data moves as fp16: the host casts x to fp16 (and the output back
to f32), halving HBM traffic vs f32 (2 x 16.8 MB per core). Gate math
stays f32 on-chip; end-to-end error is ~3e-4, far inside the 2e-2 gate.

Engine schedule (costs measured on HW for [128, 4096] fp16 chunks):
  - DVE tensor_scalar mul runs in the 4x packed mode (~1.5 us), so ALL
    scale multiplies go to DVE.
  - DVE tensor_reduce has no packed mode (1 elem/cycle, 4.3 us), so the
    pool is split: DVE tensor_add folds the two halves of a chunk
    (2x packed TT, ~1.3 us) into a [128, 2048] scratch, then ACT's
    activation-with-accum_out sums the scratch (~2.4 us) - balancing
    DVE and ACT at ~11 us/image, under the ~19 us/image DMA budget.
  - The per-image work is software-pipelined one image deep (muls for
    image b-1 are emitted after the pool/gate of image b) so DVE never
    stalls waiting on a sigmoid.

Layouts (prepared host-side):
    x      [4, 128, 4, 4096]  per-core shard, fp16. Channel c = k*128+p
                               lives at [b, p, k, :]; spatial flattened.
    w1t    [128, 4, 32]  f32   w1t[p, k, r] = w1[r, 128k + p]
    b1     [32, 1]       f32
    w2t    [32, 4, 128]  f32   w2t[r, k, p] = w2[128k + p, r]
    b2c    [128, 4]      f32   b2c[p, k]   = b2[128k + p]

All four images fit in SBUF simultaneously, so loads never wait on slot
reuse. Loads ride the Sync HWDGE queue, stores the GpSimd SWDGE queue
(a store waiting on compute never head-of-line-blocks the next load).
"""

import numpy as np

B = 32
C = 512
HW = 64 * 64
HH = HW // 2
N_CORES = 8
B_LOC = B // N_CORES
KC = C // 128  # channel chunks of 128

_NC_CACHE = {}

# Set by test harness to capture a profile; harmless default for grading.
TRACE = False
LAST_RESULT = None


def _build_nc():
    from contextlib import ExitStack

    import concourse.tile as tile
    from concourse import bacc, mybir

    f32 = mybir.dt.float32
    f16 = mybir.dt.float16
    AF = mybir.ActivationFunctionType
    nc = bacc.Bacc("TRN2", target_bir_lowering=False, debug=False)

    x = nc.dram_tensor("x", [B_LOC, 128, KC, HW], f16, kind="ExternalInput")
    w1t = nc.dram_tensor("w1t", [128, KC, 32], f32, kind="ExternalInput")
    b1 = nc.dram_tensor("b1", [32, 1], f32, kind="ExternalInput")
    w2t = nc.dram_tensor("w2t", [32, KC, 128], f32, kind="ExternalInput")
    b2c = nc.dram_tensor("b2c", [128, KC], f32, kind="ExternalInput")
    out = nc.dram_tensor("out", [B_LOC, 128, KC, HW], f16, kind="ExternalOutput")

    with ExitStack() as ctx:
        tc = ctx.enter_context(tile.TileContext(nc))
        singles = ctx.enter_context(tc.tile_pool(name="singles", bufs=1))
        xpool = ctx.enter_context(tc.tile_pool(name="xpool", bufs=B_LOC))
        halves = ctx.enter_context(tc.tile_pool(name="halves", bufs=6))
        small = ctx.enter_context(tc.tile_pool(name="small", bufs=2))
        psum = ctx.enter_context(tc.tile_pool(name="psum", bufs=2, space="PSUM"))

        w1t_sb = singles.tile([128, KC, 32], f32)
        b1_sb = singles.tile([32, 1], f32)
        w2t_sb = singles.tile([32, KC, 128], f32)
        b2_sb = singles.tile([128, KC], f32)

        xts = [None] * B_LOC
        s_tiles = [None] * B_LOC

        def emit_loads(b):
            xt = xpool.tile([128, KC, HW], f16, tag="x")
            xts[b] = xt
            if b == 0:
                # Weight loads ride the otherwise-idle SWDGE queue so
                # they never delay image loads on either HWDGE ring.
                nc.gpsimd.dma_start(out=w1t_sb, in_=w1t[:])
                nc.gpsimd.dma_start(out=b1_sb, in_=b1[:])
                nc.gpsimd.dma_start(out=w2t_sb, in_=w2t[:])
                nc.gpsimd.dma_start(out=b2_sb, in_=b2c[:])
                # Image 0 loads land per chunk so pooling starts as
                # early as possible.
                nc.sync.dma_start(out=xt[:, 0, :], in_=x[b, :, 0, :])
                nc.scalar.dma_start(out=xt[:, 2, :], in_=x[b, :, 2, :])
                nc.sync.dma_start(out=xt[:, 1, :], in_=x[b, :, 1, :])
                nc.scalar.dma_start(out=xt[:, 3, :], in_=x[b, :, 3, :])
            else:
                # A single HWDGE ring tops out at ~350 GB/s; chunks 0-1
                # ride the Sync ring and chunks 2-3 the ACT ring so the
                # two rings together saturate the ~430 GB/s fabric.
                nc.sync.dma_start(out=xt[:, 0:2, :], in_=x[b, :, 0:2, :])
                nc.scalar.dma_start(out=xt[:, 2:4, :], in_=x[b, :, 2:4, :])

        def emit_front(b):
            """Pool and gate for image b."""
            xt = xts[b]
            # Pool: DVE folds the chunk in half (packed 2x TT add),
            # ACT sums the 2048-residue via accum_out.
            sums = small.tile([128, KC], f32, tag="sums")
            for k in range(KC):
                hk = halves.tile([128, HH], f16, tag="half")
                nc.vector.tensor_add(hk, xt[:, k, 0:HH], xt[:, k, HH:HW])
                nc.scalar.activation(
                    hk, hk, AF.Copy, accum_out=sums[:, k : k + 1]
                )

            zp = psum.tile([32, 1], f32, tag="z")
            for k in range(KC):
                nc.tensor.matmul(
                    zp,
                    lhsT=w1t_sb[:, k, :],
                    rhs=sums[:, k : k + 1],
                    start=(k == 0),
                    stop=(k == KC - 1),
                )

            z = small.tile([32, 1], f32, tag="z_sb")
            nc.scalar.activation(z, zp, AF.Relu, bias=b1_sb, scale=1.0 / HW)

            sp = psum.tile([128, KC], f32, tag="s")
            for k in range(KC):
                nc.tensor.matmul(
                    sp[:, k : k + 1],
                    lhsT=w2t_sb[:, k, :],
                    rhs=z,
                    start=True,
                    stop=True,
                )
            s = small.tile([128, KC], f32, tag="s_sb")
            s_tiles[b] = s
            for k in range(KC):
                nc.scalar.activation(
                    s[:, k : k + 1], sp[:, k : k + 1], AF.Sigmoid,
                    bias=b2_sb[:, k : k + 1],
                )

        def emit_back(b):
            """Scale multiplies (DVE, 4x packed) and stores for image b.

            Stores ride the same Sync HWDGE FIFO as the loads; the
            emission order below interleaves them L0,L1,S0,L2,S1,L3,
            S2,S3 so every store is ready by its FIFO turn and the ring
            streams at fabric rate with no idle gaps.
            """
            xt, s = xts[b], s_tiles[b]
            nc.vector.tensor_scalar_mul(xt[:, 0, :], xt[:, 0, :], s[:, 0:1])
            nc.vector.tensor_scalar_mul(xt[:, 1, :], xt[:, 1, :], s[:, 1:2])
            nc.sync.dma_start(out=out[b, :, 0:2, :], in_=xt[:, 0:2, :])
            nc.vector.tensor_scalar_mul(xt[:, 2, :], xt[:, 2, :], s[:, 2:3])
            nc.vector.tensor_scalar_mul(xt[:, 3, :], xt[:, 3, :], s[:, 3:4])
            nc.scalar.dma_start(out=out[b, :, 2:4, :], in_=xt[:, 2:4, :])

        # All load triggers enqueue on the Sync FIFO before any store
        # trigger: loads stream back-to-back at fabric rate (last image
        # lands ~44us in) and the stores drain the remaining fabric time
        # with every store long ready by its FIFO turn. The compute is
        # software-pipelined one image deep (muls of image b emitted
        # after the pool/gate of image b+1) so DVE's TT adds for the
        # next image overlap the previous image's gate round-trip.
        for b in range(B_LOC):
            emit_loads(b)
        emit_front(0)
        for b in range(1, B_LOC):
            emit_front(b)
            emit_back(b - 1)
        emit_back(B_LOC - 1)

    nc.compile()
    return nc


def _get_nc():
    if "nc" not in _NC_CACHE:
        _NC_CACHE["nc"] = _build_nc()
    return _NC_CACHE["nc"]


def kernel(x, w1, b1, w2, b2):
    global LAST_RESULT
    from concourse.bass_utils import run_bass_kernel_spmd

    # [B, C, 64, 64] f32 -> [B, 128, KC, HW] fp16 with c = k*128 + p.
    xf = np.ascontiguousarray(
        x.astype(np.float16).reshape(B, KC, 128, HW).transpose(0, 2, 1, 3)
    )
    w1t = np.ascontiguousarray(w1.reshape(32, KC, 128).transpose(2, 1, 0))
    b1c = np.ascontiguousarray(b1.reshape(32, 1))
    w2t = np.ascontiguousarray(w2.reshape(KC, 128, 32).transpose(2, 0, 1))
    b2c = np.ascontiguousarray(b2.reshape(KC, 128).T)

    in_maps = [
        {
            "x": np.ascontiguousarray(xf[i * B_LOC : (i + 1) * B_LOC]),
            "w1t": w1t,
            "b1": b1c,
            "w2t": w2t,
            "b2c": b2c,
        }
        for i in range(N_CORES)
    ]

    nc = _get_nc()
    res = run_bass_kernel_spmd(
        nc, in_maps, core_ids=list(range(N_CORES)), trace=TRACE
    )
    LAST_RESULT = res
    out = np.concatenate([r["out"] for r in res.results], axis=0)
    # [B, 128, KC, HW] -> [B, KC, 128, HW] == [B, C, 64, 64]
    return (
        out.transpose(0, 2, 1, 3).reshape(B, C, 64, 64).astype(np.float32)
    )


# revision 17
# speedup vs baseline: 1.3122x; 1.0054x over previous
"""Squeeze-and-Excitation attention module on 8 Trainium2 NeuronCores.

Reference computation (per image b):
    y[c]  = mean(x[b, c, :, :])                      # global average pool
    z     = relu(w1 @ y + b1)                        # FC 512 -> 32
    s     = sigmoid(w2 @ z + b2)                     # FC 32 -> 512
    out[b, c, :, :] = x[b, c, :, :] * s[c]

Sharding: data-parallel over batch. 32 images / 8 cores = 4 images per
core; the tiny FC weights are replicated.

The kernel is HBM-bandwidth bound (read x once, write out once), so the
bulk data moves as fp16: the host casts x to fp16 (and the output back
to f32), halving HBM traffic vs f32 (2 x 16.8 MB per core). Gate math
stays f32 on-chip; end-to-end error is ~3e-4, far inside the 2e-2 gate.

Engine schedule (costs measured on HW for [128, 4096] fp16 chunks):
  - DVE tensor_scalar mul runs in the 4x packed mode (~1.5 us), so ALL
    scale multiplies go to DVE.
  - DVE tensor_reduce has no packed mode (1 elem/cycle, 4.3 us), so the
    pool is split: DVE tensor_add folds the two halves of a chunk
    (2x packed TT, ~1.2 us) into a [128, 2048] scratch, then ACT's
    activation-with-accum_out sums the scratch (~2.0 us) - balancing
    DVE and ACT at ~10 us/image, under the ~19 us/image DMA budget.
  - The per-image work is software-pipelined one image deep (muls for
    image b-1 are emitted after the pool/gate of image b) so DVE never
    stalls waiting on a sigmoid.

DMA: one HWDGE ring tops out at ~350 GB/s, and the 16 SDMA engines cap
aggregate HBM<->SBUF traffic at ~425 GB/s, so the bulk traffic is split
across BOTH HWDGE rings (Sync ring: chunks 0-1, ACT ring: chunks 2-3),
each ring FIFO-ordered loads-ahead-of-stores so a store never delays a
load and the rings stream the full 2 x 16.78 MB without idling.
Measured on hardware: ~93 us per core (vs 170 us for the all-f32
single-pass baseline): ~7 us Tile preamble, ~80 us fabric-saturated
DMA, ~3 us final store completion + exit barrier.

Layouts (prepared host-side):
    x      [4, 128, 4, 4096]  per-core shard, fp16. Channel c = k*128+p
                               lives at [b, p, k, :]; spatial flattened.
    w1t    [128, 4, 32]  f32   w1t[p, k, r] = w1[r, 128k + p]
    b1     [32, 1]       f32
    w2t    [32, 4, 128]  f32   w2t[r, k, p] = w2[128k + p, r]
    b2c    [128, 4]      f32   b2c[p, k]   = b2[128k + p]

All four images fit in SBUF simultaneously, so loads never wait on slot
reuse. Loads ride the Sync HWDGE queue, stores the GpSimd SWDGE queue
(a store waiting on compute never head-of-line-blocks the next load).
"""

import numpy as np

B = 32
C = 512
HW = 64 * 64
HH = HW // 2
N_CORES = 8
B_LOC = B // N_CORES
KC = C // 128  # channel chunks of 128

_NC_CACHE = {}

# Set by test harness to capture a profile; harmless default for grading.
TRACE = False
LAST_RESULT = None


def _build_nc():
    from contextlib import ExitStack

    import concourse.tile as tile
    from concourse import bacc, mybir

    f32 = mybir.dt.float32
    f16 = mybir.dt.float16
    AF = mybir.ActivationFunctionType
    nc = bacc.Bacc("TRN2", target_bir_lowering=False, debug=False)

    x = nc.dram_tensor("x", [B_LOC, 128, KC, HW], f16, kind="ExternalInput")
    w1t = nc.dram_tensor("w1t", [128, KC, 32], f32, kind="ExternalInput")
    b1 = nc.dram_tensor("b1", [32, 1], f32, kind="ExternalInput")
    w2t = nc.dram_tensor("w2t", [32, KC, 128], f32, kind="ExternalInput")
    b2c = nc.dram_tensor("b2c", [128, KC], f32, kind="ExternalInput")
    out = nc.dram_tensor("out", [B_LOC, 128, KC, HW], f16, kind="ExternalOutput")

    with ExitStack() as ctx:
        tc = ctx.enter_context(tile.TileContext(nc))
        singles = ctx.enter_context(tc.tile_pool(name="singles", bufs=1))
        xpool = ctx.enter_context(tc.tile_pool(name="xpool", bufs=B_LOC))
        halves = ctx.enter_context(tc.tile_pool(name="halves", bufs=6))
        small = ctx.enter_context(tc.tile_pool(name="small", bufs=2))
        psum = ctx.enter_context(tc.tile_pool(name="psum", bufs=2, space="PSUM"))

        w1t_sb = singles.tile([128, KC, 32], f32)
        b1_sb = singles.tile([32, 1], f32)
        w2t_sb = singles.tile([32, KC, 128], f32)
        b2_sb = singles.tile([128, KC], f32)

        xts = [None] * B_LOC
        s_tiles = [None] * B_LOC

        def emit_loads(b):
            xt = xpool.tile([128, KC, HW], f16, tag="x")
            xts[b] = xt
            if b == 0:
                # Weight loads ride the otherwise-idle SWDGE queue so
                # they never delay image loads on either HWDGE ring.
                nc.gpsimd.dma_start(out=w1t_sb, in_=w1t[:])
                nc.gpsimd.dma_start(out=b1_sb, in_=b1[:])
                nc.gpsimd.dma_start(out=w2t_sb, in_=w2t[:])
                nc.gpsimd.dma_start(out=b2_sb, in_=b2c[:])
                # Image 0 loads land per chunk so pooling starts as
                # early as possible.
                nc.sync.dma_start(out=xt[:, 0, :], in_=x[b, :, 0, :])
                nc.scalar.dma_start(out=xt[:, 2, :], in_=x[b, :, 2, :])
                nc.sync.dma_start(out=xt[:, 1, :], in_=x[b, :, 1, :])
                nc.scalar.dma_start(out=xt[:, 3, :], in_=x[b, :, 3, :])
            else:
                # A single HWDGE ring tops out at ~350 GB/s; chunks 0-1
                # ride the Sync ring and chunks 2-3 the ACT ring so the
                # two rings together saturate the ~430 GB/s fabric.
                nc.sync.dma_start(out=xt[:, 0:2, :], in_=x[b, :, 0:2, :])
                nc.scalar.dma_start(out=xt[:, 2:4, :], in_=x[b, :, 2:4, :])

        def emit_front(b):
            """Pool and gate for image b."""
            xt = xts[b]
            # Pool: DVE folds the chunk in half (packed 2x TT add),
            # ACT sums the 2048-residue via accum_out.
            sums = small.tile([128, KC], f32, tag="sums")
            for k in range(KC):
                hk = halves.tile([128, HH], f16, tag="half")
                nc.vector.tensor_add(hk, xt[:, k, 0:HH], xt[:, k, HH:HW])
                nc.scalar.activation(
                    hk, hk, AF.Copy, accum_out=sums[:, k : k + 1]
                )

            zp = psum.tile([32, 1], f32, tag="z")
            for k in range(KC):
                nc.tensor.matmul(
                    zp,
                    lhsT=w1t_sb[:, k, :],
                    rhs=sums[:, k : k + 1],
                    start=(k == 0),
                    stop=(k == KC - 1),
                )

            z = small.tile([32, 1], f32, tag="z_sb")
            nc.scalar.activation(z, zp, AF.Relu, bias=b1_sb, scale=1.0 / HW)

            sp = psum.tile([128, KC], f32, tag="s")
            for k in range(KC):
                nc.tensor.matmul(
                    sp[:, k : k + 1],
                    lhsT=w2t_sb[:, k, :],
                    rhs=z,
                    start=True,
                    stop=True,
                )
            s = small.tile([128, KC], f32, tag="s_sb")
            s_tiles[b] = s
            for k in range(KC):
                nc.scalar.activation(
                    s[:, k : k + 1], sp[:, k : k + 1], AF.Sigmoid,
                    bias=b2_sb[:, k : k + 1],
                )

        def emit_back(b):
            """Scale multiplies (DVE, 4x packed) and stores for image b.

            Stores ride the same Sync HWDGE FIFO as the loads; the
            emission order below interleaves them L0,L1,S0,L2,S1,L3,
            S2,S3 so every store is ready by its FIFO turn and the ring
            streams at fabric rate with no idle gaps.
            """
            xt, s = xts[b], s_tiles[b]
            nc.vector.tensor_scalar_mul(xt[:, 0, :], xt[:, 0, :], s[:, 0:1])
            nc.vector.tensor_scalar_mul(xt[:, 1, :], xt[:, 1, :], s[:, 1:2])
            nc.sync.dma_start(out=out[b, :, 0:2, :], in_=xt[:, 0:2, :])
            nc.vector.tensor_scalar_mul(xt[:, 2, :], xt[:, 2, :], s[:, 2:3])
            nc.vector.tensor_scalar_mul(xt[:, 3, :], xt[:, 3, :], s[:, 3:4])
            nc.scalar.dma_start(out=out[b, :, 2:4, :], in_=xt[:, 2:4, :])

        # Per ring the FIFO runs L0 L1 L2 S0 L3 S1 S2 S3: loads lead so
        # no store ever delays a load, and each store is emitted late
        # enough that it is ready by its FIFO turn - the rings never
        # idle between the load phase and the store drain. The compute
        # is software-pipelined one image deep (muls of image b emitted
        # after the pool/gate of image b+1) so DVE's TT adds for the
        # next image overlap the previous image's gate round-trip.
        emit_loads(0)
        emit_loads(1)
        emit_loads(2)
        emit_front(0)
        emit_front(1)
        emit_back(0)
        emit_loads(3)
        emit_front(2)
        emit_back(1)
        emit_front(3)
        emit_back(2)
        emit_back(3)

    nc.compile()
    return nc


def _get_nc():
    if "nc" not in _NC_CACHE:
        _NC_CACHE["nc"] = _build_nc()
    return _NC_CACHE["nc"]


def kernel(x, w1, b1, w2, b2):
    global LAST_RESULT
    from concourse.bass_utils import run_bass_kernel_spmd

    # [B, C, 64, 64] f32 -> [B, 128, KC, HW] fp16 with c = k*128 + p.
    xf = np.ascontiguousarray(
        x.astype(np.float16).reshape(B, KC, 128, HW).transpose(0, 2, 1, 3)
    )
    w1t = np.ascontiguousarray(w1.reshape(32, KC, 128).transpose(2, 1, 0))
    b1c = np.ascontiguousarray(b1.reshape(32, 1))
    w2t = np.ascontiguousarray(w2.reshape(KC, 128, 32).transpose(2, 0, 1))
    b2c = np.ascontiguousarray(b2.reshape(KC, 128).T)

    in_maps = [
        {
            "x": np.ascontiguousarray(xf[i * B_LOC : (i + 1) * B_LOC]),
            "w1t": w1t,
            "b1": b1c,
            "w2t": w2t,
            "b2c": b2c,
        }
        for i in range(N_CORES)
    ]

    nc = _get_nc()
    res = run_bass_kernel_spmd(
        nc, in_maps, core_ids=list(range(N_CORES)), trace=TRACE
    )
    LAST_RESULT = res
    out = np.concatenate([r["out"] for r in res.results], axis=0)
    # [B, 128, KC, HW] -> [B, KC, 128, HW] == [B, C, 64, 64]
    return (
        out.transpose(0, 2, 1, 3).reshape(B, C, 64, 64).astype(np.float32)
    )
